# revision 14
# baseline (speedup 1.0000x reference)
"""Trainium2 Bass kernel for nn_CovarianceLayer: local 5x5 covariance of two images.

reference:
    xc = x[:, :, 2:-2, 2:-2]; yc likewise
    x_dev = xc - box5x5(x)/25 ; y_dev = yc - box5x5(y)/25
    out   = box5x5(x_dev * y_dev)/25            # [B,1,1016,1016]

Strategy (pure data parallel over batch, 2 images per NeuronCore, 8 cores):
  Host stages inputs as fp16 row-interleaved [1024, 4096] (one 1MB HWDGE
  load per block, 8KB contiguous per partition) and reads back fp16
  outputs [1016, 2032] (upcast on host). fp16 keeps rel err ~6e-4,
  ~30x under the 2e-2 gate, and halves HBM traffic.
  Per 128-row block (stride 120, 8-row vertical halo):
    - horizontal 5-tap box sums on DVE via a custom prefix-scan op
      over all 4 panes at once (fp16); seed sums on ScalarE accum_out
    - vertical 5-tap conv + center-crop subtraction fused into PE fp16
      matmuls: PSUM(2-bank tile per image) = Wid^T @ xy - (band/32)^T @ hx
    - ScalarE drains PSUM -> fp16 SBUF, one 1020-wide copy per (tensor,img)
    - p = xd * yd on DVE (fp16 2x mode)
    - final 2D box split to balance engines: img0 via 5 column-shifted
      accumulating PE matmuls; img1 via DVE box-scan + one PE band matmul
    - fp16 stores on the sync HWDGE ring (keeps ScalarE free for drains);
      block 0's load is split x/y so the first scan starts earlier
"""

import numpy as np

import concourse.bacc as bacc
import concourse.mybir as mybir
import concourse.tile as tile
import concourse.dve_ops as dve_ops
from concourse.dve_spec import Spec, Src0, Src1, C0, scan, AluOp, lower
from concourse.dve_uop import DveOpSpec
from concourse.dve_ops import DveOp
from concourse import bass_utils

dt = mybir.dt

H = W = 1024
HO = WO = 1016   # output spatial dims
HD = WD = 1020   # x_dev dims
B_PER_CORE = 2
N_CORES = 8
BLK = 120        # output rows per block
N_BLOCKS = (HO + BLK - 1) // BLK  # 9


def _register_box5():
    """out[p,k] = sum_{d=0..4} v[p,k+d]; in0=v[:,4:4+N], in1=v[:,0:N], s0=sum(v[:,0:4])."""
    name = "BOX5_ANT"
    for op in dve_ops.OPS:
        if op.name == name:
            return op
    body = scan(AluOp.ADD, Src0 - Src1, init=C0) + Src1

    def ref(in0, in1, c0, c1, c2):
        return np.cumsum(in0 - in1, axis=-1, dtype=np.float32) + in1 + c0

    spec = Spec(body=body, reference=ref)
    row = dve_ops._CUSTOM_DVE_ROW_BASE + len(dve_ops.OPS)
    shas = {}
    for ver in ("v3", "v4"):
        uops = lower(spec, ver=ver)
        shas[ver] = DveOpSpec(name=name, opcode=row, uops=uops, rd1_en=True).sha(ver)
    op = DveOp(name, spec, subdim=False, uops_sha=shas)
    dve_ops.OPS.append(op)
    dve_ops.CUSTOM_DVE_SPECS[name] = spec
    dve_ops._SUB_OPCODE_FOR_NAME[name] = row
    return op


# psum_dev = (25/32)*xc - box2D(x)/32 = (25/32) * x_dev   (exact in fp16)
# p' = (25/32)^2 * p;  psum_out = sum_{k,d} p'/16;  out = psum_out * OUT_SCALE
WID_V = 25.0 / 32.0
OUT_SCALE = 16.0 / (25.0 * WID_V * WID_V)  # = 1.048576 (exact)


def _make_weights():
    # Wid[k, m] = 25/32 iff k == m+2   (center-crop tap)
    # Wnb[k, m] = -1/32 iff m <= k <= m+4  (negated vertical band)
    # Wpb[k, m] = +1/16 iff m <= k <= m+4  (final vertical band)
    wid = np.zeros((128, 128), dtype=np.float32)
    wnb = np.zeros((128, 128), dtype=np.float32)
    for m in range(124):
        wid[m + 2, m] = WID_V
        wnb[m:m + 5, m] = -1.0 / 32.0
    wpb = np.zeros((124, 128), dtype=np.float32)
    for m in range(120):
        wpb[m:m + 5, m] = 1.0 / 16.0
    return (wid.astype(np.float16), wnb.astype(np.float16),
            wpb.astype(np.float16))


def block_geom(i):
    r0 = BLK * i
    return (r0, min(128, H - r0), min(124, HD - r0), min(BLK, HO - r0))


def build_bass():
    box5 = _register_box5()
    nc = bacc.Bacc("TRN2", target_bir_lowering=False)

    PW = W + 4   # padded pane stride: [4 zeros | 1024 data] per pane, host-packed
    xy_d = nc.dram_tensor("xy", [H, 4 * PW], dt.float16, kind="ExternalInput")
    zz_d = nc.dram_tensor("zz", [128, 1], dt.float32, kind="ExternalInput")
    wid_d = nc.dram_tensor("wid", [128, 128], dt.float16, kind="ExternalInput")
    wnb_d = nc.dram_tensor("wnb", [128, 128], dt.float16, kind="ExternalInput")
    wpb_d = nc.dram_tensor("wpb", [124, 128], dt.float16, kind="ExternalInput")
    o_d = nc.dram_tensor("o", [HO, 2 * WO], dt.float16, kind="ExternalOutput")

    with tile.TileContext(nc) as tc:
        with tc.tile_pool(name="wts", bufs=1) as wts, \
             tc.tile_pool(name="sbA", bufs=4) as sbA, \
             tc.tile_pool(name="sbB", bufs=3) as sbB, \
             tc.tile_pool(name="ps_dev", bufs=3, space="PSUM") as ps_dev, \
             tc.tile_pool(name="ps_out", bufs=2, space="PSUM") as ps_out:

            # weight loads on the scalar ring so the first block load is
            # at the head of the sync ring
            wid_t = wts.tile([128, 128], dt.float16)
            nc.scalar.dma_start(wid_t[:], wid_d[:])
            wnb_t = wts.tile([128, 128], dt.float16)
            nc.scalar.dma_start(wnb_t[:], wnb_d[:])
            wpb_t = wts.tile([124, 128], dt.float16)
            nc.scalar.dma_start(wpb_t[:], wpb_d[:])
            zeros_t = wts.tile([128, 1], dt.float32)
            nc.scalar.dma_start(zeros_t[:], zz_d[:])

            state = {}

            def phase0(i):
                r0, rows, _, _ = block_geom(i)
                xy_t = sbA.tile([128, 4 * PW], dt.float16, tag="xy_t")
                if i == 0:
                    # per-pane loads so the first scans start sooner
                    for p in range(4):
                        nc.sync.dma_start(xy_t[0:rows, p * PW:(p + 1) * PW],
                                          xy_d[r0:r0 + rows, p * PW:(p + 1) * PW])
                else:
                    nc.sync.dma_start(xy_t[0:rows, :], xy_d[r0:r0 + rows, :])
                state[("xy", i)] = xy_t

            def phase1(i):
                r0, rows, d_rows, _ = block_geom(i)
                xy_t = state.pop(("xy", i))

                hxy = sbB.tile([128, 4 * PW - 4], dt.float16, tag="hxy")
                if i == 0:
                    # per-pane scans chase the per-pane loads
                    for p in range(4):
                        nc.vector._custom_dve(box5, out=hxy[0:rows, p * PW:p * PW + W],
                                              in0=xy_t[0:rows, p * PW + 4:(p + 1) * PW],
                                              in1=xy_t[0:rows, p * PW:p * PW + W],
                                              s0=zeros_t[0:rows, :])
                else:
                    # one BOX5 covers all four panes; the host-packed zero pads
                    # make init=0 exact at every pane start
                    nc.vector._custom_dve(box5, out=hxy[0:rows, 0:4 * PW - 4],
                                          in0=xy_t[0:rows, 4:4 * PW],
                                          in1=xy_t[0:rows, 0:4 * PW - 4],
                                          s0=zeros_t[0:rows, :])

                xd_s = sbB.tile([128, 2 * WD], dt.float16, tag="xd_s")
                yd_s = sbB.tile([128, 2 * WD], dt.float16, tag="yd_s")
                # per tensor: two 2-bank psum tiles (one per image); centers
                # share stationary wid, bands share wnb; one wide drain per img
                for t, dst in ((0, xd_s), (1, yd_s)):
                    tiles = []
                    for di in range(2):
                        src0 = (2 * t + di) * PW + 4
                        ps_t = ps_dev.tile([128, 1024], dt.float32, tag="devps")
                        tiles.append((ps_t, src0, di))
                    for ps_t, src0, di in tiles:
                        for c0, cn in ((0, 512), (512, WD - 512)):
                            nc.tensor.matmul(ps_t[:, c0:c0 + cn],
                                             lhsT=wid_t[0:rows, :],
                                             rhs=xy_t[0:rows, src0 + 2 + c0:src0 + 2 + c0 + cn],
                                             start=True, stop=False)
                    for ps_t, src0, di in tiles:
                        for c0, cn in ((0, 512), (512, WD - 512)):
                            nc.tensor.matmul(ps_t[:, c0:c0 + cn],
                                             lhsT=wnb_t[0:rows, :],
                                             rhs=hxy[0:rows, src0 + c0:src0 + c0 + cn],
                                             start=False, stop=True)
                    for ps_t, src0, di in tiles:
                        nc.scalar.copy(dst[0:d_rows, di * WD:di * WD + WD],
                                       ps_t[0:d_rows, 0:WD])
                state[i] = (xd_s, yd_s)

            def phase23(i):
                r0, _, d_rows, o_rows = block_geom(i)
                xd_s, yd_s = state.pop(i)
                # p = xd * yd on DVE (fp16 2x mode), both images in one op
                p_s = sbA.tile([128, 2 * WD], dt.float16, tag="p_s")
                nc.vector.tensor_mul(p_s[0:d_rows, :],
                                     xd_s[0:d_rows, :], yd_s[0:d_rows, :])
                o_s = sbA.tile([128, 2 * WO], dt.float16, tag="o_s")
                # img0: final 2D box fully on PE -- 5 column-shifted
                # accumulating band matmuls per colgroup (shared stationary)
                for c0, cn in ((0, 512), (512, WO - 512)):
                    out_ps = ps_out.tile([128, 512], dt.float32, tag="out_ps")
                    for dlt in range(5):
                        nc.tensor.matmul(out_ps[:, 0:cn],
                                         lhsT=wpb_t[0:d_rows, :],
                                         rhs=p_s[0:d_rows, c0 + dlt:c0 + dlt + cn],
                                         start=(dlt == 0), stop=(dlt == 4))
                    nc.scalar.activation(o_s[0:o_rows, c0:c0 + cn],
                                         out_ps[0:o_rows, 0:cn],
                                         mybir.ActivationFunctionType.Copy,
                                         scale=OUT_SCALE)
                # img1: horizontal box on DVE scan, then one band matmul
                s3p = sbB.tile([128, 1], dt.float32, tag="s3p")
                nc.vector.tensor_reduce(s3p[0:d_rows, :], p_s[0:d_rows, WD:WD + 4],
                                        op=mybir.AluOpType.add, axis=mybir.AxisListType.X)
                hp = sbB.tile([128, WO], dt.float16, tag="hp")
                nc.vector._custom_dve(box5, out=hp[0:d_rows, 0:WO],
                                      in0=p_s[0:d_rows, WD + 4:2 * WD],
                                      in1=p_s[0:d_rows, WD:WD + WO],
                                      s0=s3p[0:d_rows, :])
                for c0, cn in ((0, 512), (512, WO - 512)):
                    out_ps = ps_out.tile([128, 512], dt.float32, tag="out_ps")
                    nc.tensor.matmul(out_ps[:, 0:cn],
                                     lhsT=wpb_t[0:d_rows, :],
                                     rhs=hp[0:d_rows, c0:c0 + cn],
                                     start=True, stop=True)
                    nc.scalar.activation(o_s[0:o_rows, WO + c0:WO + c0 + cn],
                                         out_ps[0:o_rows, 0:cn],
                                         mybir.ActivationFunctionType.Copy,
                                         scale=OUT_SCALE)
                nc.gpsimd.dma_start(o_d[r0:r0 + o_rows, :], o_s[0:o_rows, :])

            # software-pipelined emission: loads 2 ahead of phase1; the
            # mul/final-conv/store trail phase1 by 2. phase23 is emitted
            # before phase1 so per-engine FIFO order matches readiness.
            for it in range(N_BLOCKS + 4):
                if it < N_BLOCKS:
                    phase0(it)
                if 4 <= it < N_BLOCKS + 4:
                    phase23(it - 4)
                if 2 <= it < N_BLOCKS + 2:
                    phase1(it - 2)

    nc.compile()
    return nc


_NC = None


def _get_nc():
    global _NC
    if _NC is None:
        _NC = build_bass()
    return _NC


def kernel(x: np.ndarray, y: np.ndarray, mean_mask: np.ndarray = None, *,
           trace: bool = False, **_ignored):
    """Full inputs x,y [16,1,1024,1024] f32 -> full output [16,1,1016,1016] f32."""
    assert x.shape == (16, 1, H, W) and y.shape == (16, 1, H, W)
    nc = _get_nc()
    wid, wnb, wpb = _make_weights()
    x4 = x[:, 0].astype(np.float16)
    y4 = y[:, 0].astype(np.float16)
    in_maps = []
    for c in range(N_CORES):
        b0 = c * B_PER_CORE
        # row-interleaved pack with 4 zero pad columns before each pane:
        # row r -> [0000 x_img0[r] | 0000 x_img1[r] | 0000 y_img0[r] | 0000 y_img1[r]]
        xy = np.zeros((H, 4 * (W + 4)), dtype=np.float16)
        for p, pane in enumerate((x4[b0], x4[b0 + 1], y4[b0], y4[b0 + 1])):
            xy[:, p * (W + 4) + 4:(p + 1) * (W + 4)] = pane
        in_maps.append({
            "xy": xy,
            "wid": wid, "wnb": wnb, "wpb": wpb,
            "zz": np.zeros((128, 1), dtype=np.float32),
        })
    kw = {}
    if trace:
        kw = dict(trace=True, trace_cores=[0])
    res = bass_utils.run_bass_kernel_spmd(nc, in_maps, core_ids=list(range(N_CORES)),
                                          **kw)
    outs = []
    for r in res.results:
        o = r["o"].astype(np.float32)   # [1016, 2032]: [img0 | img1]
        outs.append(o[:, :WO])
        outs.append(o[:, WO:])
    kernel.last_results = res
    return np.stack(outs, axis=0).reshape(16, 1, HO, WO)
